# revision 46
# baseline (speedup 1.0000x reference)
"""Trainium2 Bass kernel for BasicInteractionNetworkModule.

Data-parallel over batch (B=16) across 8 NeuronCores, 2 batches/core.

Math (per batch b):
  senders   = S^T @ O          [R, 128]   (S = sender_relations [128, R])
  receivers = R_rel^T @ O      [R, 128]
  rel_x = [senders, receivers, info]   [R, 320]
  h = relu-MLP(rel_x): 320 -> 256 -> 256 -> 256 -> 128 (relu after every layer)
  eff_recv = R_rel @ effects   [128, 128]
  obj_x = [O, ext, eff_recv]   [128, 288]
  out = relu-MLP2(obj_x): 288 -> 256 -> 256 -> 128 (no final relu)

Device strategy: feature-major activations (partition = feature) so every
relation-MLP layer is out^T = W^T @ H^T with stationary weights; bias+ReLU
land on ACT/DVE per-partition. Layer-1 folds the sender/receiver projection
via A_s = O @ rw1[:128], A_r = O @ rw1[128:256] so S and R stream from DRAM
as the moving operand. The info term is padded to K=128 (zero weight rows +
zeroed moving partitions 64-127) so all L1 matmuls are uniform full-row
matmuls and LDWEIGHTS pull-ahead never stalls on row-group conflicts.
L4's free-dim bias is folded out entirely: E3 = max(psum, -b4) on DVE, and
the missing +b4 is restored in the aggregation as the rank-1 term
b4 (x) rowsum(R_rel), with rowsum(R) precomputed on host.
The two batches run as one flat software-pipelined chunk stream; batch 0's
object MLP is emitted under batch 1's relation pipeline.
"""

import numpy as np
import ml_dtypes

B, N_OBJ, N_REL = 16, 128, 16256
OBJ_D, REL_D, EFF_D, EXT_D, OUT_D = 128, 64, 128, 32, 128
HID = 256
N_CORES = 8
B_CORE = B // N_CORES  # 2
M_CHUNK = 1024

_CACHE = {}


def _chunks():
    out = []
    base = 0
    while base < N_REL:
        mc = min(M_CHUNK, N_REL - base)
        out.append((base, mc))
        base += mc
    return out


def _mtiles(mc):
    out = []
    base = 0
    while base < mc:
        n = min(512, mc - base)
        out.append((base, n))
        base += n
    return out


def build_kernel():
    from concourse import bacc
    import concourse.mybir as mybir
    import concourse.tile as tile

    F32 = mybir.dt.float32
    F32R = mybir.dt.float32r
    BF16 = mybir.dt.bfloat16
    RELU = mybir.ActivationFunctionType.Relu
    ADD = mybir.AluOpType.add
    MAX = mybir.AluOpType.max
    MULT = mybir.AluOpType.mult

    nc = bacc.Bacc(None)

    # per-core inputs. S, R and the (partition-duplicated) info strips are
    # host-packed per chunk into one tensor so each chunk is a single
    # contiguous 6KB-per-partition DMA.
    NCHUNK = (N_REL + M_CHUNK - 1) // M_CHUNK
    C_d = nc.dram_tensor("sri", [B_CORE, 128, NCHUNK, 3 * M_CHUNK], BF16,
                         kind="ExternalInput")
    # receiver^T pre-arranged as [128, N_REL/128, N_OBJ] per batch so each
    # chunk's DMA is one contiguous 2KB-per-partition transfer (256B rows of
    # the naive layout run below SDMA line-rate and throttled the queue)
    RT_d = nc.dram_tensor("r_rel_t4", [B_CORE, 128, N_REL // 128, N_OBJ], BF16,
                          kind="ExternalInput")
    OTb_d = nc.dram_tensor("objs_tb", [B_CORE, OBJ_D, N_OBJ], BF16, kind="ExternalInput")
    XTb_d = nc.dram_tensor("ext_tb", [B_CORE, EXT_D, N_OBJ], BF16, kind="ExternalInput")
    rsR_d = nc.dram_tensor("rs_recv", [B_CORE, 128, N_OBJ], F32, kind="ExternalInput")

    rw1s_d = nc.dram_tensor("rw1s", [128, HID], BF16, kind="ExternalInput")
    rw1r_d = nc.dram_tensor("rw1r", [128, HID], BF16, kind="ExternalInput")
    # info weights padded to K=128: rows 64-127 are zero
    rw1if_d = nc.dram_tensor("rw1if", [128, 2, 128], BF16, kind="ExternalInput")
    rw2_d = nc.dram_tensor("rw2f", [128, 2, HID], BF16, kind="ExternalInput")
    rw3_d = nc.dram_tensor("rw3f", [128, 2, HID], BF16, kind="ExternalInput")
    rw4_d = nc.dram_tensor("rw4b", [128, 2, EFF_D], BF16, kind="ExternalInput")
    ow1o_d = nc.dram_tensor("ow1o", [128, HID], BF16, kind="ExternalInput")
    ow1x_d = nc.dram_tensor("ow1x", [EXT_D, HID], BF16, kind="ExternalInput")
    ow1e_d = nc.dram_tensor("ow1e", [128, HID], BF16, kind="ExternalInput")
    ow2_d = nc.dram_tensor("ow2f", [128, 2, HID], BF16, kind="ExternalInput")
    ow3_d = nc.dram_tensor("ow3f", [128, 2, OUT_D], BF16, kind="ExternalInput")

    rb1_d = nc.dram_tensor("rb1c", [128, 2], F32, kind="ExternalInput")
    rb2_d = nc.dram_tensor("rb2c", [128, 2], F32, kind="ExternalInput")
    rb3_d = nc.dram_tensor("rb3c", [128, 2], F32, kind="ExternalInput")
    ob1_d = nc.dram_tensor("ob1c", [128, 2], F32, kind="ExternalInput")
    ob2_d = nc.dram_tensor("ob2c", [128, 2], F32, kind="ExternalInput")
    ob3_d = nc.dram_tensor("ob3r", [128, OUT_D], F32, kind="ExternalInput")
    negb4_d = nc.dram_tensor("negb4", [128, 512], BF16, kind="ExternalInput")
    b4col_d = nc.dram_tensor("b4col", [128, 1], F32, kind="ExternalInput")

    out_d = nc.dram_tensor("out", [B_CORE, N_OBJ, OUT_D], F32, kind="ExternalOutput")

    NCH = len(_chunks())  # 16 chunks per batch
    allc = [(b, base, mc) for b in range(B_CORE) for (base, mc) in _chunks()]
    NTOT = len(allc)

    with tile.TileContext(nc) as tc:
        with (
            tc.tile_pool(name="wts", bufs=1) as wts,
            tc.tile_pool(name="perb", bufs=2) as perb,
            tc.tile_pool(name="cin", bufs=4) as cin,
            tc.tile_pool(name="acts", bufs=3) as acts,
            tc.tile_pool(name="psB", bufs=3, space="PSUM") as psB,
            tc.tile_pool(name="psS", bufs=2, space="PSUM") as psS,
        ):
            # ---- persistent weights ----
            rw1s = wts.tile([128, HID], BF16)
            rw1r = wts.tile([128, HID], BF16)
            rw1if = wts.tile([128, 2, 128], BF16)
            rw2 = wts.tile([128, 2, HID], BF16)
            rw3 = wts.tile([128, 2, HID], BF16)
            rw4 = wts.tile([128, 2, EFF_D], BF16)
            ow1o = wts.tile([128, HID], BF16)
            ow1x = wts.tile([EXT_D, HID], BF16)
            ow1e = wts.tile([128, HID], BF16)
            ow2 = wts.tile([128, 2, HID], BF16)
            ow3 = wts.tile([128, 2, OUT_D], BF16)
            rb1 = wts.tile([128, 2], F32)
            rb2 = wts.tile([128, 2], F32)
            rb3 = wts.tile([128, 2], F32)
            ob1 = wts.tile([128, 2], F32)
            ob2 = wts.tile([128, 2], F32)
            ob3 = wts.tile([128, OUT_D], F32)
            negb4 = wts.tile([128, 512], BF16)
            b4col = wts.tile([128, 1], F32)
            rsR = [wts.tile([128, N_OBJ], F32, tag=f"rsR{b}", name=f"rsR{b}")
                   for b in range(B_CORE)]
            sts = [dict() for _ in range(NTOT)]

            def dma_issue(i):
                """allocate + fetch chunk i's streaming inputs (sync queue)"""
                b, base, mc = allc[i]
                st = sts[i]
                if 'C_c' in st:
                    return
                ns = mc // 128
                C_c = cin.tile([128, 3 * M_CHUNK], BF16, tag="C_c", name="C_c")
                RT_c = cin.tile([128, M_CHUNK // 128, N_OBJ], BF16, tag="RT_c",
                                name="RT_c")
                if i == 0:
                    # split the first chunk so L1's operands land in need
                    # order instead of behind one 768KB transfer
                    for lo, hi in ((0, M_CHUNK), (M_CHUNK, 2 * M_CHUNK),
                                   (2 * M_CHUNK, 3 * M_CHUNK)):
                        nc.sync.dma_start(C_c[:, lo:hi], C_d[b, :, 0, lo:hi])
                else:
                    nc.sync.dma_start(C_c, C_d[b, :, base // M_CHUNK, :])
                s0 = base // 128
                nc.sync.dma_start(RT_c[:, :ns, :], RT_d[b, :, s0:s0 + ns, :])
                st['S_c'] = C_c[:, 0:M_CHUNK]
                st['R_c'] = C_c[:, M_CHUNK:2 * M_CHUNK]
                st['I_c'] = C_c[:, 2 * M_CHUNK:3 * M_CHUNK]
                st['C_c'] = C_c; st['RT_c'] = RT_c

            # ---- startup: interleave chunk-0 inputs with the critical
            # weight fetches on the sync queue so L1(0) starts ASAP ----
            OTb0 = perb.tile([OBJ_D, N_OBJ], BF16, tag="OTb0")
            OTb1 = perb.tile([OBJ_D, N_OBJ], BF16, tag="OTb1")
            # pre-warm the PE's HAM clock gate during the dead startup window
            # (instruction-fetch + first DMAs): ~4us of dummy matmuls on a
            # zeroed scratch tile flip the clock to 2.4GHz before real work
            warm = wts.tile([128, 512], BF16)
            nc.vector.memset(warm, 0.0)
            psW = psS.tile([128, 512], F32, tag="sm", name="psW")
            for _ in range(15):
                nc.tensor.matmul(psW, warm[:, 0:128], warm, start=True, stop=True)
            nc.sync.dma_start(OTb0, OTb_d[0])
            nc.sync.dma_start(rw1s, rw1s_d[:])
            nc.sync.dma_start(rw1r, rw1r_d[:])
            nc.sync.dma_start(OTb1, OTb_d[1])
            dma_issue(0)

            # everything else on the gpsimd (SWDGE) queue; order = need time
            nc.gpsimd.dma_start(rw1if, rw1if_d[:])
            nc.gpsimd.dma_start(rb1, rb1_d[:])
            nc.gpsimd.dma_start(rw2, rw2_d[:])
            nc.gpsimd.dma_start(rb2, rb2_d[:])
            nc.gpsimd.dma_start(rw3, rw3_d[:])
            nc.gpsimd.dma_start(rb3, rb3_d[:])
            nc.gpsimd.dma_start(rw4, rw4_d[:])
            nc.gpsimd.dma_start(negb4, negb4_d[:])
            nc.gpsimd.dma_start(b4col, b4col_d[:])
            for b in range(B_CORE):
                nc.gpsimd.dma_start(rsR[b], rsR_d[b])
            for t, d in [(ow1o, ow1o_d), (ow1x, ow1x_d), (ow1e, ow1e_d),
                         (ow2, ow2_d), (ow3, ow3_d),
                         (ob1, ob1_d), (ob2, ob2_d), (ob3, ob3_d)]:
                nc.gpsimd.dma_start(t, d[:])

            OTbs = [OTb0, OTb1]
            XTbs, Ass, Ars, effTs, effTbs = [], [], [], [], []
            for b in range(B_CORE):
                XTb = perb.tile([EXT_D, N_OBJ], BF16, tag=f"XTb{b}", name=f"XTb{b}")
                nc.gpsimd.dma_start(XTb, XTb_d[b])
                XTbs.append(XTb)
                As = perb.tile([N_OBJ, HID], BF16, tag=f"As{b}", name=f"As{b}")
                Ar = perb.tile([N_OBJ, HID], BF16, tag=f"Ar{b}", name=f"Ar{b}")
                Ass.append(As); Ars.append(Ar)
                effT = perb.tile([EFF_D, N_OBJ], F32, tag=f"effT{b}", name=f"effT{b}")
                effTs.append(effT)
                effTb = perb.tile([EFF_D, N_OBJ], BF16, tag=f"effTb{b}",
                                  name=f"effTb{b}")
                effTbs.append(effTb)

            def setup_batch(b):
                psA = psS.tile([128, HID], F32, tag="sm", name="psA")
                nc.tensor.matmul(psA, OTbs[b], rw1s, start=True, stop=True)
                nc.vector.tensor_copy(Ass[b], psA)
                psA2 = psS.tile([128, HID], F32, tag="sm", name="psA2")
                nc.tensor.matmul(psA2, OTbs[b], rw1r, start=True, stop=True)
                nc.vector.tensor_copy(Ars[b], psA2)

            setup_batch(0)

            def stageA1(i):
                """chunk inputs + L1 + H1 evac. Matmul order per hid-chunk is
                S,S,R,R,I,I: same-stationary pairs minimize LDWEIGHTS churn,
                and the chunk-opening As load pipelines under the preceding
                N=128 L4/agg region."""
                b, base, mc = allc[i]
                st = sts[i]
                As, Ar = Ass[b], Ars[b]
                dma_issue(i)
                S_c, R_c, I_c = st['S_c'], st['R_c'], st['I_c']
                H1 = acts.tile([128, 2, M_CHUNK], BF16, tag="H1")
                for p2 in range(2):
                    ps = psB.tile([128, M_CHUNK], F32, tag="ps")
                    h = slice(p2 * 128, (p2 + 1) * 128)
                    tiles = _mtiles(mc)
                    for mt, n in tiles:
                        sl = slice(mt, mt + n)
                        nc.tensor.matmul(ps[:, sl], As[:, h], S_c[:, sl], start=True, stop=False)
                    for mt, n in tiles:
                        sl = slice(mt, mt + n)
                        nc.tensor.matmul(ps[:, sl], Ar[:, h], R_c[:, sl], start=False, stop=False)
                    for mt, n in tiles:
                        sl = slice(mt, mt + n)
                        nc.tensor.matmul(ps[:, sl], rw1if[:, p2, :], I_c[:, sl], start=False, stop=True)
                    nc.scalar.activation(H1[:, p2, :mc], ps[:, :mc], RELU,
                                         bias=rb1[:, p2:p2 + 1], scale=1.0)
                st['H1'] = H1

            def stageA2(i):
                """L2 + H2 evac"""
                b, base, mc = allc[i]
                st = sts[i]
                H1 = st['H1']
                H2 = acts.tile([128, 2, M_CHUNK], BF16, tag="H2")
                for p2 in range(2):
                    ps = psB.tile([128, M_CHUNK], F32, tag="ps")
                    h = slice(p2 * 128, (p2 + 1) * 128)
                    for mt, n in _mtiles(mc):
                        sl = slice(mt, mt + n)
                        nc.tensor.matmul(ps[:, sl], rw2[:, 0, h], H1[:, 0, sl], start=True, stop=False)
                    for mt, n in _mtiles(mc):
                        sl = slice(mt, mt + n)
                        nc.tensor.matmul(ps[:, sl], rw2[:, 1, h], H1[:, 1, sl], start=False, stop=True)
                    nc.scalar.activation(H2[:, p2, :mc], ps[:, :mc], RELU,
                                         bias=rb2[:, p2:p2 + 1], scale=1.0)
                st['H2'] = H2

            def stageB1(i):
                """L3 + H3 evac (DVE)"""
                b, base, mc = allc[i]
                st = sts[i]
                H2 = st['H2']
                H3 = acts.tile([128, 2, M_CHUNK], BF16, tag="H3")
                for p2 in range(2):
                    ps = psB.tile([128, M_CHUNK], F32, tag="ps")
                    h = slice(p2 * 128, (p2 + 1) * 128)
                    for mt, n in _mtiles(mc):
                        sl = slice(mt, mt + n)
                        nc.tensor.matmul(ps[:, sl], rw3[:, 0, h], H2[:, 0, sl], start=True, stop=False)
                    for mt, n in _mtiles(mc):
                        sl = slice(mt, mt + n)
                        nc.tensor.matmul(ps[:, sl], rw3[:, 1, h], H2[:, 1, sl], start=False, stop=True)
                    # p2=0 evacuates on ACT so DVE (which also carries the
                    # effT accumulate) isn't the serial bottleneck for L4
                    if p2 == 0:
                        nc.scalar.activation(H3[:, 0, :mc], ps[:, :mc], RELU,
                                             bias=rb3[:, 0:1], scale=1.0)
                    else:
                        nc.vector.tensor_scalar(H3[:, 1, :mc], ps[:, :mc],
                                                rb3[:, 1:2], 0.0, ADD, MAX)
                st['H3'] = H3

            def stageB2(i):
                """L4 (no bias matmul) + E3 = max(psum, -b4) evac (DVE)"""
                b, base, mc = allc[i]
                st = sts[i]
                ns = mc // 128
                H3 = st['H3']
                E3 = acts.tile([128, M_CHUNK], BF16, tag="E3")
                for g in range(0, ns, 4):
                    ge = min(g + 4, ns)
                    span = (ge - g) * 128
                    gsl = slice(g * 128, g * 128 + span)
                    ps4 = psS.tile([128, 512], F32, tag="sm")
                    # k-major: all k0 matmuls first so they can run while
                    # H3[:,1,:] is still being evacuated
                    for sj in range(g, ge):
                        sl = slice(sj * 128, (sj + 1) * 128)
                        psl = slice((sj - g) * 128, (sj - g + 1) * 128)
                        nc.tensor.matmul(ps4[:, psl], H3[:, 0, sl], rw4[:, 0, :],
                                         start=(sj == g), stop=False)
                    for sj in range(g, ge):
                        sl = slice(sj * 128, (sj + 1) * 128)
                        psl = slice((sj - g) * 128, (sj - g + 1) * 128)
                        nc.tensor.matmul(ps4[:, psl], H3[:, 1, sl], rw4[:, 1, :],
                                         start=False, stop=(sj == ge - 1))
                    nc.vector.tensor_tensor(E3[:, gsl], ps4[:, :span],
                                            negb4[:, :span], MAX)
                st['E3'] = E3

            def stageAgg(i):
                """aggregate effects for chunk i into effT[b]; on the first
                chunk of a batch, seed with the rank-1 bias term
                b4 (x) rowsum(R)."""
                b, base, mc = allc[i]
                st = sts[i]
                ns = mc // 128
                E3 = st['E3']; RT_c = st['RT_c']
                effT = effTs[b]
                psa = psS.tile([128, N_OBJ], F32, tag="sm")
                for sj in range(ns):
                    nc.tensor.matmul(psa, E3[:, sj * 128:(sj + 1) * 128],
                                     RT_c[:, sj, :],
                                     start=(sj == 0), stop=(sj == ns - 1))
                if i % NCH == 0:
                    nc.vector.scalar_tensor_tensor(effT, rsR[b], b4col, psa,
                                                   MULT, ADD)
                elif i % NCH == NCH - 1:
                    # final chunk: write the completed sum straight to bf16
                    # for the object MLP (skips a separate downcast hop)
                    nc.vector.tensor_tensor(effTbs[b], effT, psa, ADD)
                else:
                    nc.vector.tensor_tensor(effT, effT, psa, ADD)
                st.clear()

            def objG1(b, pp):
                OTb, XTb, effTb = OTbs[b], XTbs[b], effTbs[b]
                G1 = perb.tile([128, 2, N_OBJ], BF16, tag="G1")
                for p2 in range(2):
                    ps = pp.tile([128, N_OBJ], F32, tag="sm" if pp is psS else "ps",
                                 name="psg1")
                    h = slice(p2 * 128, (p2 + 1) * 128)
                    nc.tensor.matmul(ps, ow1o[:, h], OTb, start=True, stop=False)
                    nc.tensor.matmul(ps, ow1x[:, h], XTb, start=False, stop=False)
                    nc.tensor.matmul(ps, ow1e[:, h], effTb, start=False, stop=True)
                    nc.scalar.activation(G1[:, p2, :], ps, RELU,
                                         bias=ob1[:, p2:p2 + 1], scale=1.0)
                return G1

            def objG2(b, G1, pp):
                G2 = perb.tile([128, 2, N_OBJ], BF16, tag="G2")
                for p2 in range(2):
                    ps = pp.tile([128, N_OBJ], F32, tag="sm" if pp is psS else "ps",
                                 name="psg2")
                    h = slice(p2 * 128, (p2 + 1) * 128)
                    nc.tensor.matmul(ps, ow2[:, 0, h], G1[:, 0, :], start=True, stop=False)
                    nc.tensor.matmul(ps, ow2[:, 1, h], G1[:, 1, :], start=False, stop=True)
                    nc.scalar.activation(G2[:, p2, :], ps, RELU,
                                         bias=ob2[:, p2:p2 + 1], scale=1.0)
                return G2

            def objOut(b, G2, pp):
                pso = pp.tile([N_OBJ, OUT_D], F32, tag="sm" if pp is psS else "ps",
                              name="pso")
                nc.tensor.matmul(pso, G2[:, 0, :], ow3[:, 0, :], start=True, stop=False)
                nc.tensor.matmul(pso, G2[:, 1, :], ow3[:, 1, :], start=False, stop=True)
                ob = perb.tile([N_OBJ, OUT_D], F32, tag="ob")
                nc.vector.tensor_tensor(ob, pso, ob3, ADD)
                nc.sync.dma_start(out_d[b], ob)

            # ---- flat software pipeline over both batches' chunks ----
            # iteration i emits: L1(i) | agg(i-2) | L3(i-1) | L2(i) | L4(i-1)
            # so every consumer has a full matmul block between it and its
            # producer's PSUM evacuation. Batch 0's object MLP is emitted
            # under batch 1's pipeline right after effT[0] completes.
            obj_state = {}
            for i in range(NTOT):
                stageA1(i)
                if i == 1:
                    setup_batch(1)
                if i >= 2:
                    stageAgg(i - 2)
                    done = i - 2
                    if done == NCH - 1:
                        obj_state['G1'] = objG1(0, psS)
                    elif done == NCH:
                        obj_state['G2'] = objG2(0, obj_state.pop('G1'), psS)
                    elif done == NCH + 1:
                        objOut(0, obj_state.pop('G2'), psS)
                if i >= 1:
                    stageB1(i - 1)
                stageA2(i)
                if i >= 1:
                    stageB2(i - 1)
            stageAgg(NTOT - 2)
            stageB1(NTOT - 1)
            stageB2(NTOT - 1)
            stageAgg(NTOT - 1)
            G1 = objG1(1, psB)
            G2 = objG2(1, G1, psB)
            objOut(1, G2, psB)

    nc.compile()
    return nc


def _prep_inputs(objects, sender_relations, receiver_relations, relation_info,
                 external_effect_info, rw1, rb1, rw2, rb2, rw3, rb3, rw4, rb4,
                 ow1, ob1, ow2, ob2, ow3, ob3):
    bf16 = ml_dtypes.bfloat16
    f32 = np.float32

    def a(x):
        return np.ascontiguousarray(np.asarray(x, dtype=f32))

    objects = a(objects); sender_relations = a(sender_relations)
    receiver_relations = a(receiver_relations); relation_info = a(relation_info)
    external_effect_info = a(external_effect_info)
    rw1, rb1, rw2, rb2, rw3, rb3, rw4, rb4 = map(a, (rw1, rb1, rw2, rb2, rw3, rb3, rw4, rb4))
    ow1, ob1, ow2, ob2, ow3, ob3 = map(a, (ow1, ob1, ow2, ob2, ow3, ob3))

    info_t_bf = np.ascontiguousarray(relation_info.transpose(0, 2, 1)).astype(bf16)
    s_bf = sender_relations.astype(bf16)
    r_bf = receiver_relations.astype(bf16)

    # pack [S | R | info-dup] per 1024-relation chunk: one contiguous
    # 6KB-per-partition DMA per chunk. Tail chunk zero-padded to 1024.
    NCHUNK = (N_REL + M_CHUNK - 1) // M_CHUNK
    NPAD = NCHUNK * M_CHUNK
    def _pad(x):
        out = np.zeros(x.shape[:-1] + (NPAD,), dtype=bf16)
        out[..., :N_REL] = x
        return out.reshape(x.shape[0], x.shape[1], NCHUNK, M_CHUNK)
    s4 = _pad(s_bf)
    r4 = _pad(r_bf)
    i4 = _pad(info_t_bf)
    i4 = np.concatenate([i4, i4], axis=1)  # duplicate across partition halves
    sri = np.ascontiguousarray(np.concatenate([s4, r4, i4], axis=3))
    # receiver^T laid out [B, 128, N_REL/128, N_OBJ]: partition p of chunk s
    # holds relation s*128+p, contiguous in s for one 2KB DMA per partition
    r_rel_t4 = np.ascontiguousarray(
        receiver_relations.transpose(0, 2, 1)
        .reshape(B, N_REL // 128, 128, N_OBJ)
        .transpose(0, 2, 1, 3)).astype(bf16)
    objs_t = np.ascontiguousarray(objects.transpose(0, 2, 1))
    ext_t = np.ascontiguousarray(external_effect_info.transpose(0, 2, 1))

    # K=128-padded info weights: rows 64-127 zero
    rw1if = np.zeros((128, 2, 128), dtype=bf16)
    rw1if[0:64, 0, :] = rw1[256:320, 0:128].astype(bf16)
    rw1if[0:64, 1, :] = rw1[256:320, 128:256].astype(bf16)

    # L4 bias folded out of the matmul path: E3 = max(psum, -b4bf) on device,
    # then effT += b4bf (x) rowsum(R_bf16). Exact cancellation requires the
    # same bf16-rounded b4 in both places.
    b4bf = rb4.astype(bf16)
    negb4 = np.ascontiguousarray(np.tile(-b4bf[None, :], (128, 4)))
    b4col = np.ascontiguousarray(b4bf.astype(f32).reshape(128, 1))
    rowsum = r_bf.astype(f32).sum(axis=2)  # [B, N_OBJ]
    rs_bcast = np.ascontiguousarray(
        np.broadcast_to(rowsum[:, None, :], (B, 128, N_OBJ)).astype(f32))

    shared = {
        "rw1s": rw1[0:128].astype(bf16),
        "rw1r": rw1[128:256].astype(bf16),
        "rw1if": rw1if,
        "rw2f": np.ascontiguousarray(rw2.reshape(2, 128, HID).transpose(1, 0, 2)).astype(bf16),
        "rw3f": np.ascontiguousarray(rw3.reshape(2, 128, HID).transpose(1, 0, 2)).astype(bf16),
        "rw4b": np.ascontiguousarray(rw4.reshape(2, 128, EFF_D).transpose(1, 0, 2)).astype(bf16),
        "ow1o": ow1[0:128].astype(bf16),
        "ow1x": ow1[128:160].astype(bf16),
        "ow1e": ow1[160:288].astype(bf16),
        "ow2f": np.ascontiguousarray(ow2.reshape(2, 128, HID).transpose(1, 0, 2)).astype(bf16),
        "ow3f": np.ascontiguousarray(ow3.reshape(2, 128, OUT_D).transpose(1, 0, 2)).astype(bf16),
        "rb1c": np.ascontiguousarray(rb1.reshape(2, 128).T),
        "rb2c": np.ascontiguousarray(rb2.reshape(2, 128).T),
        "rb3c": np.ascontiguousarray(rb3.reshape(2, 128).T),
        "ob1c": np.ascontiguousarray(ob1.reshape(2, 128).T),
        "ob2c": np.ascontiguousarray(ob2.reshape(2, 128).T),
        "ob3r": np.ascontiguousarray(np.broadcast_to(ob3[None, :], (128, OUT_D))),
        "negb4": negb4,
        "b4col": b4col,
    }

    in_maps = []
    for c in range(N_CORES):
        sl = slice(c * B_CORE, (c + 1) * B_CORE)
        m = dict(shared)
        m["sri"] = sri[sl]
        m["r_rel_t4"] = r_rel_t4[sl]
        m["objs_tb"] = objs_t[sl].astype(bf16)
        m["ext_tb"] = ext_t[sl].astype(bf16)
        m["rs_recv"] = rs_bcast[sl]
        in_maps.append(m)
    return in_maps


def run(in_maps, **spmd_kwargs):
    from concourse.bass_utils import run_bass_kernel_spmd

    if "nc" not in _CACHE:
        _CACHE["nc"] = build_kernel()
    return run_bass_kernel_spmd(_CACHE["nc"], in_maps,
                                core_ids=list(range(N_CORES)), **spmd_kwargs)


def kernel(**inputs) -> np.ndarray:
    in_maps = _prep_inputs(**inputs)
    res = run(in_maps)
    out = np.concatenate([r["out"].reshape(-1, OUT_D) for r in res.results], axis=0)
    return np.ascontiguousarray(out, dtype=np.float32)
